# revision 1
# baseline (speedup 1.0000x reference)
"""Trainium2 Bass kernel for a dense transformer block, sharded over 8 NeuronCores.

Sharding: core c handles batch b=c//2 and half hf=c%2 of that batch's 2048
tokens ("own" tokens). K/V are computed for the full 2048-token batch on both
cores of a pair, so no collectives are needed.

v2: fp16 matmul path (fp32 PSUM accumulation), everything SBUF-resident (no
DRAM round-trip for h), each weight matrix streamed exactly once, exp done in
[128,1024] chunks to amortize ACT overhead.
"""

import numpy as np

from contextlib import ExitStack

import concourse.bass as bass
import concourse.bacc as bacc
import concourse.tile as tile
import concourse.mybir as mybir

F32 = mybir.dt.float32
F32R = mybir.dt.float32r
F16 = mybir.dt.float16
AF = mybir.ActivationFunctionType
OP = mybir.AluOpType

EPS = 1e-5

PHASE_MARKS = []


def _mark(nc, name):
    PHASE_MARKS.append((name, int(nc.get_next_instruction_name()[2:])))


class Cfg:
    def __init__(self, E=1024, H=16, MLP=4096, T_OWN=1024, T_FULL=2048, repeat=1,
                 skip=()):
        self.E, self.H, self.MLP = E, H, MLP
        self.T_OWN, self.T_FULL = T_OWN, T_FULL
        self.D = E // H
        self.NE = E // 128          # feature tiles
        self.NM = MLP // 128        # mlp feature tiles
        self.NQB = T_OWN // 512     # own-token 512-blocks
        self.NFB = T_FULL // 512    # full-token 512-blocks
        self.NTK = T_FULL // 128    # full-token 128-blocks (k positions)
        self.G = 2                  # head groups
        self.HPG = H // self.G      # heads per group
        self.NP_G = self.HPG // 2   # head-pairs per group
        self.repeat = repeat
        self.skip = frozenset(skip)


def build(cfg: Cfg):
    E, MLP, T_OWN, T_FULL = cfg.E, cfg.MLP, cfg.T_OWN, cfg.T_FULL

    nc = bacc.Bacc("TRN2", target_bir_lowering=False, debug=False)

    d = {}
    d["xT"] = nc.dram_tensor("xT", [E, T_FULL], F16, kind="ExternalInput")
    d["qkvT"] = nc.dram_tensor("qkvT", [E, 3 * E], F16, kind="ExternalInput")
    d["fcT"] = nc.dram_tensor("fcT", [E, E], F16, kind="ExternalInput")
    d["w1T"] = nc.dram_tensor("w1T", [E, MLP], F16, kind="ExternalInput")
    d["w2T"] = nc.dram_tensor("w2T", [MLP, E], F16, kind="ExternalInput")
    d["ln1"] = nc.dram_tensor("ln1", [2, E], F32, kind="ExternalInput")
    d["ln2"] = nc.dram_tensor("ln2", [2, E], F32, kind="ExternalInput")
    d["fcb"] = nc.dram_tensor("fcb", [E], F32, kind="ExternalInput")
    d["b1"] = nc.dram_tensor("b1", [MLP], F32, kind="ExternalInput")
    d["b2"] = nc.dram_tensor("b2", [E], F32, kind="ExternalInput")
    d["ones"] = nc.dram_tensor("ones", [T_FULL], F32, kind="ExternalInput")
    d["out"] = nc.dram_tensor("out", [E, T_OWN], F32, kind="ExternalOutput")

    PHASE_MARKS.clear()
    with tile.TileContext(nc) as tc, nc.allow_low_precision(
        reason="fp16 matmul inputs by design"
    ):
        if cfg.repeat == 1:
            _body(nc, tc, cfg, d)
        else:
            with tc.For_i(0, cfg.repeat, 1, hint_engines=(
                    mybir.EngineType.PE, mybir.EngineType.Activation,
                    mybir.EngineType.DVE, mybir.EngineType.SP)):
                _body(nc, tc, cfg, d)
    nc.compile()
    return nc


def _ln_stats(nc, cfg, pools, src_fn, nblk, ones_col, eps_t, srst, snb):
    """Column stats over the feature dim via ones-matmuls (fp16 inputs).

    src_fn(e, tb) -> [128,512] F16 AP; writes rstd into srst[0:1] (f32) and
    -mu*rstd into snb[0:1] (f32)."""
    E, NE = cfg.E, cfg.NE
    sq_pool, st_ps, row_pool = pools
    for tb in range(nblk):
        sl = slice(tb * 512, (tb + 1) * 512)
        s1 = st_ps.tile([1, 512], F32, tag="s1")
        s2 = st_ps.tile([1, 512], F32, tag="s2")
        for e in range(NE):
            src = src_fn(e, tb)
            sq = sq_pool.tile([128, 512], F16, tag="sq")
            nc.vector.tensor_tensor(sq[:], src, src, OP.mult)
            nc.tensor.matmul(s1[:], ones_col[:], src, start=(e == 0), stop=(e == NE - 1))
            nc.tensor.matmul(s2[:], ones_col[:], sq[:], start=(e == 0), stop=(e == NE - 1))
        m_row = row_pool.tile([1, 512], F32, tag="mrow")
        nc.vector.tensor_scalar_mul(m_row[:], s1[:], 1.0 / E)
        v_row = row_pool.tile([1, 512], F32, tag="vrow")
        nc.vector.tensor_scalar_mul(v_row[:], s2[:], 1.0 / E)
        msq = row_pool.tile([1, 512], F32, tag="msq")
        nc.vector.tensor_tensor(msq[:], m_row[:], m_row[:], OP.mult)
        nc.vector.tensor_tensor(v_row[:], v_row[:], msq[:], OP.subtract)
        sd = row_pool.tile([1, 512], F32, tag="sd")
        nc.scalar.activation(sd[:], v_row[:], AF.Sqrt, bias=eps_t[:], scale=1.0)
        nc.vector.reciprocal(srst[0:1, sl], sd[:])
        nc.vector.scalar_tensor_tensor(
            snb[0:1, sl], m_row[:], -1.0, srst[0:1, sl].bitcast(F32),
            op0=OP.mult, op1=OP.mult)


def _ln_apply(nc, map_ps, gb_ap, srst, snb, src_ap, dst_ap, sl):
    """dst(F16) = src * (g x rstd) + (g x (-mu*rstd) + b x 1), all [128,512].

    gb_ap: [2,128] f32 AP (rows g, b) for this feature tile."""
    a_ps = map_ps.tile([128, 512], F32, tag="amap")
    nc.tensor.matmul(a_ps[:], gb_ap[0:1, :],
                     srst[0:1, sl], start=True, stop=True)
    b_ps = map_ps.tile([128, 512], F32, tag="bmap")
    nc.tensor.matmul(b_ps[:], gb_ap[0:2, :],
                     snb[0:2, sl], start=True, stop=True)
    nc.vector.tensor_tensor(dst_ap, src_ap, a_ps[:], OP.mult)
    nc.vector.tensor_tensor(dst_ap, dst_ap, b_ps[:], OP.add)


def _body(nc, tc, cfg, d):
    E, H, MLP, D = cfg.E, cfg.H, cfg.MLP, cfg.D
    NE, NM, NQB, NFB, NTK = cfg.NE, cfg.NM, cfg.NQB, cfg.NFB, cfg.NTK
    T_OWN, T_FULL, G, HPG, NP_G = cfg.T_OWN, cfg.T_FULL, cfg.G, cfg.HPG, cfg.NP_G
    xT, qkvT, fcT, w1T, w2T = d["xT"], d["qkvT"], d["fcT"], d["w1T"], d["w2T"]
    ln1, ln2, fcb, b1, b2 = d["ln1"], d["ln2"], d["fcb"], d["b1"], d["b2"]
    ones, out = d["ones"], d["out"]

    with ExitStack() as ctx:
        consts = ctx.enter_context(tc.tile_pool(name="consts", bufs=1))

        ones_col = consts.tile([128, 1], F16)
        nc.vector.memset(ones_col[:], 1.0)
        ones64 = consts.tile([1, 64], F32R)
        nc.sync.dma_start(ones64[:], ones.ap()[0:64].unsqueeze(0).bitcast(F32R))
        eps_t = consts.tile([1, 1], F32)
        nc.vector.memset(eps_t[:], EPS)
        ln1t = consts.tile([2, E], F32R)
        nc.sync.dma_start(ln1t[:], ln1.ap().bitcast(F32R))
        ln2t = consts.tile([2, E], F32R)
        nc.sync.dma_start(ln2t[:], ln2.ap().bitcast(F32R))
        fcb_t = consts.tile([128, NE], F32)
        nc.sync.dma_start(fcb_t[:], fcb.ap().rearrange("(a p) -> p a", p=128))
        b1_t = consts.tile([128, NM], F32)
        nc.sync.dma_start(b1_t[:], b1.ap().rearrange("(a p) -> p a", p=128))
        b2_t = consts.tile([128, NE], F32)
        nc.sync.dma_start(b2_t[:], b2.ap().rearrange("(a p) -> p a", p=128))

        # ================= Phases 1-3: LN1, QKV, attention =================
        with ExitStack() as p1:
            q_pool = p1.enter_context(tc.tile_pool(name="qp", bufs=NE))
            q_tiles = [q_pool.tile([128, T_OWN], F16, tag="qt", name="qt")
                       for _ in range(NE)]
            k_pool = p1.enter_context(tc.tile_pool(name="kp", bufs=G * NP_G))
            v_pool = p1.enter_context(tc.tile_pool(name="vp", bufs=G * NTK))
            k_tiles, v_tiles = {}, {}

            # --- Phase A: fused LN1 + K/Q projections (per token block) ---
            _mark(nc, "A:ln1")
            hstk = ExitStack()
            h_pool = hstk.enter_context(tc.tile_pool(name="hp", bufs=NE))
            h_tiles = [h_pool.tile([128, T_FULL], F16, tag="ht", name="ht")
                       for _ in range(NE)]
            with ExitStack() as pA:
                xf_pool = pA.enter_context(tc.tile_pool(name="xfp", bufs=2 * NE))
                sq_pool = pA.enter_context(tc.tile_pool(name="sqp", bufs=4))
                st_ps = pA.enter_context(tc.tile_pool(name="stps", bufs=1, space="PSUM"))
                row_pool = pA.enter_context(tc.tile_pool(name="rows", bufs=2))
                map_ps = pA.enter_context(tc.tile_pool(name="mapps", bufs=1, space="PSUM"))
                stat_pool = pA.enter_context(tc.tile_pool(name="statp", bufs=1))
                wq_pool = pA.enter_context(tc.tile_pool(name="wqp", bufs=NE))
                wk_pool = pA.enter_context(tc.tile_pool(name="wkp", bufs=2 * NE))
                acc_ps = pA.enter_context(tc.tile_pool(name="accps", bufs=4, space="PSUM"))
                srst1 = stat_pool.tile([1, T_FULL], F32R, tag="srst1")
                snb1 = stat_pool.tile([2, T_FULL], F32R, tag="snb1")
                nc.sync.dma_start(snb1[1:2, :], ones.ap()[0:T_FULL].unsqueeze(0).bitcast(F32R))

                xts0 = []
                for e in range(NE):
                    t = xf_pool.tile([128, 512], F16, tag="xf", name="xf")
                    eng = nc.sync if e % 2 == 0 else nc.scalar
                    eng.dma_start(t[:], xT.ap()[e * 128:(e + 1) * 128, 0:512])
                    xts0.append(t)
                wq = []
                for e in range(NE):
                    t = wq_pool.tile([128, E], F16, tag="wq", name="wq")
                    nc.scalar.dma_start(t[:], qkvT.ap()[e * 128:(e + 1) * 128, 0:E])
                    wq.append(t)
                wk = {}
                for g in range(G):
                    for e in range(NE):
                        t = wk_pool.tile([128, HPG * D], F16, tag="wk", name="wk")
                        col0 = E + g * HPG * D
                        nc.scalar.dma_start(
                            t[:], qkvT.ap()[e * 128:(e + 1) * 128, col0:col0 + HPG * D])
                        wk[(g, e)] = t
                for g in range(G):
                    for dkt in range(NP_G):
                        k_tiles[(g, dkt)] = k_pool.tile([128, T_FULL], F16,
                                                        tag="kt", name="kt")

                for tb in range(NFB):
                    sl = slice(tb * 512, (tb + 1) * 512)
                    if tb == 0:
                        xts = xts0
                    else:
                        xts = []
                        for e in range(NE):
                            t = xf_pool.tile([128, 512], F16, tag="xf", name="xf")
                            nc.sync.dma_start(t[:], xT.ap()[e * 128:(e + 1) * 128, sl])
                            xts.append(t)
                    _ln_stats(nc, cfg, (sq_pool, st_ps, row_pool),
                              lambda e, _tb: xts[e][:], 1, ones_col, eps_t,
                              srst1[0:1, sl], snb1[0:2, sl])
                    for e in range(NE):
                        _ln_apply(nc, map_ps, ln1t[:, e * 128:(e + 1) * 128],
                                  srst1, snb1, xts[e][:], h_tiles[e][:, sl], sl)
                    # K projections for this token block (both groups)
                    if "kv" not in cfg.skip:
                        for g in range(G):
                            for dkt in range(NP_G):
                                ps = acc_ps.tile([128, 512], F32, tag="acc", name="acc")
                                for e in range(NE):
                                    nc.tensor.matmul(
                                        ps[:], wk[(g, e)][:, dkt * 128:(dkt + 1) * 128],
                                        h_tiles[e][:, sl],
                                        start=(e == 0), stop=(e == NE - 1))
                                nc.vector.tensor_copy(k_tiles[(g, dkt)][:, sl], ps[:])
                    # Q projections (own token blocks only)
                    if "q" not in cfg.skip and tb < NQB:
                        for eg in range(NE):
                            ps = acc_ps.tile([128, 512], F32, tag="acc", name="acc")
                            for e in range(NE):
                                nc.tensor.matmul(
                                    ps[:], wq[e][:, eg * 128:(eg + 1) * 128],
                                    h_tiles[e][:, sl],
                                    start=(e == 0), stop=(e == NE - 1))
                            nc.vector.tensor_copy(q_tiles[eg][:, sl], ps[:])

            # --- Phase B2: V projection (full tokens), both groups ---
            _mark(nc, "B2:kv")
            with ExitStack() as pkv:
                wv_pool = pkv.enter_context(tc.tile_pool(name="wvp", bufs=2))
                kv_ps = pkv.enter_context(tc.tile_pool(name="kvps", bufs=4, space="PSUM"))
                for g in (() if "kv" in cfg.skip else range(G)):
                    wv = []
                    for e in range(NE):
                        t = wv_pool.tile([128, HPG * D], F16, tag=f"wv{e}", name="wv")
                        col0 = 2 * E + g * HPG * D
                        nc.scalar.dma_start(
                            t[:], qkvT.ap()[e * 128:(e + 1) * 128, col0:col0 + HPG * D])
                        wv.append(t)
                    # V: [kpos, head, 65] tiles; col 64 = ones (denominator trick)
                    for tk in range(NTK):
                        vt = v_pool.tile([128, HPG, 65], F16, tag="vt", name="vt")
                        v_tiles[(g, tk)] = vt
                        nc.vector.memset(vt[:, :, 64:65], 1.0)
                        ps = kv_ps.tile([128, HPG * D], F32, tag="kvacc", name="kvacc")
                        off = tk * 128
                        for e in range(NE):
                            nc.tensor.matmul(ps[:], h_tiles[e][:, off:off + 128],
                                             wv[e][:],
                                             start=(e == 0), stop=(e == NE - 1))
                        nc.vector.tensor_copy(
                            vt[:, :, 0:64], ps[:].rearrange("p (h dd) -> p h dd", dd=D))

            hstk.close()  # h freed; attention does not need it

            # --- Phase C: attention, per group ---
            _mark(nc, "C:att")
            av_pool = ctx.enter_context(tc.tile_pool(name="avp", bufs=NE, side="right"))
            av_tiles = [av_pool.tile([128, T_OWN], F16, tag="avt", name="avt")
                        for _ in range(NE)]
            # prefetch fc weights + residual x while attention runs (DMA idle)
            xo_pool = ctx.enter_context(tc.tile_pool(name="xop", bufs=NE, side="right"))
            wf_pool = ctx.enter_context(tc.tile_pool(name="wfp", bufs=NE, side="right"))
            xo = []
            for e in range(NE):
                t = xo_pool.tile([128, T_OWN], F16, tag="xo", name="xo")
                nc.sync.dma_start(t[:], xT.ap()[e * 128:(e + 1) * 128, 0:T_OWN])
                xo.append(t)
            wf = []
            for e in range(NE):
                t = wf_pool.tile([128, E], F16, tag="wf", name="wf")
                nc.scalar.dma_start(t[:], fcT.ap()[e * 128:(e + 1) * 128, :])
                wf.append(t)
            if "att" in cfg.skip:
                for t in av_tiles:
                    nc.vector.memset(t[:, 0:1], 0.0)
            for g in (() if "att" in cfg.skip else range(G)):
                with ExitStack() as pa:
                    sc_ps = pa.enter_context(
                        tc.tile_pool(name=f"scps{g}", bufs=2, space="PSUM"))
                    av_ps = pa.enter_context(
                        tc.tile_pool(name=f"avps{g}", bufs=2, space="PSUM"))
                    ex_pool = pa.enter_context(tc.tile_pool(name=f"exp{g}", bufs=8))
                    rec_pool = pa.enter_context(tc.tile_pool(name=f"rec{g}", bufs=6))
                    for hp in range(NP_G):
                        hpg = g * NP_G + hp
                        av_a = av_ps.tile([65, T_OWN], F32, tag="av", name="av")
                        av_b = av_ps.tile([65, T_OWN], F32, tag="av", name="av")
                        kt = k_tiles[(g, hp)]
                        for tk in range(NTK):
                            ksl = slice(tk * 128, (tk + 1) * 128)
                            sc_a = sc_ps.tile([128, T_OWN], F32, tag="sc", name="sc")
                            sc_b = sc_ps.tile([128, T_OWN], F32, tag="sc", name="sc")
                            for qh in range(NQB):
                                qsl = slice(qh * 512, (qh + 1) * 512)
                                # rows 0-63 and 64-127 are disjoint row-groups:
                                # adjacent matmuls run concurrently on the PE
                                nc.tensor.matmul(sc_a[:, qsl], kt[0:64, ksl],
                                                 q_tiles[hpg][0:64, qsl],
                                                 start=True, stop=True)
                                nc.tensor.matmul(sc_b[:, qsl], kt[64:128, ksl],
                                                 q_tiles[hpg][64:128, qsl],
                                                 start=True, stop=True)
                            ex_a = ex_pool.tile([128, T_OWN], F16, tag="ex", name="ex")
                            ex_b = ex_pool.tile([128, T_OWN], F16, tag="ex", name="ex")
                            if "expcopy" in cfg.skip:
                                nc.vector.tensor_copy(ex_a[:], sc_a[:])
                                nc.vector.tensor_copy(ex_b[:], sc_b[:])
                            elif "exp512" in cfg.skip:
                                for qh in range(NQB):
                                    qsl = slice(qh * 512, (qh + 1) * 512)
                                    nc.scalar.activation(ex_a[:, qsl], sc_a[:, qsl], AF.Exp)
                                    nc.scalar.activation(ex_b[:, qsl], sc_b[:, qsl], AF.Exp)
                            else:
                                nc.scalar.activation(ex_a[:], sc_a[:], AF.Exp)
                                nc.scalar.activation(ex_b[:], sc_b[:], AF.Exp)
                            for head, ex_t, av_t in ((0, ex_a, av_a), (1, ex_b, av_b)):
                                vslc = v_tiles[(g, tk)][:, 2 * hp + head, :]
                                for qh in range(NQB):
                                    qsl = slice(qh * 512, (qh + 1) * 512)
                                    nc.tensor.matmul(av_t[:, qsl], vslc, ex_t[:, qsl],
                                                     start=(tk == 0), stop=(tk == NTK - 1))
                        for head, av_t in ((0, av_a), (1, av_b)):
                            rrow = rec_pool.tile([1, T_OWN], F32R, tag="rr", name="rr")
                            nc.vector.reciprocal(rrow[:], av_t[64:65, :])
                            rm = sc_ps.tile([64, T_OWN], F32, tag="sc", name="rm")
                            for qh in range(NQB):
                                qsl = slice(qh * 512, (qh + 1) * 512)
                                nc.tensor.matmul(rm[:, qsl], ones64[:],
                                                 rrow[0:1, qsl],
                                                 start=True, stop=True)
                            rms = rec_pool.tile([64, T_OWN], F32, tag="rms", name="rms")
                            nc.vector.tensor_copy(rms[:], rm[:])
                            nc.vector.tensor_tensor(
                                av_tiles[hpg][head * 64:(head + 1) * 64, :],
                                av_t[0:64, :], rms[:], OP.mult)
        # h/q/k/v freed here

        # ================= Phase 4: fc_out + residual =================
        _mark(nc, "D:fc")
        x2_pool = ctx.enter_context(tc.tile_pool(name="x2p", bufs=NE))
        x2_tiles = [x2_pool.tile([128, T_OWN], F16, tag="x2t", name="x2t")
                    for _ in range(NE)]
        with ExitStack() as p4:
            fc_ps = p4.enter_context(tc.tile_pool(name="fcps", bufs=6, space="PSUM"))
            if "fc" in cfg.skip:
                for t in x2_tiles:
                    nc.vector.memset(t[:, 0:1], 0.0)
            for og in (() if "fc" in cfg.skip else range(NE // 2)):
                ps = {(j, qh): fc_ps.tile([128, 512], F32, tag="fc", name="fc")
                      for j in range(2) for qh in range(NQB)}
                for e in range(NE):
                    for j in range(2):
                        o = og * 2 + j
                        for qh in range(NQB):
                            nc.tensor.matmul(
                                ps[(j, qh)][:], wf[e][:, o * 128:(o + 1) * 128],
                                av_tiles[e][:, qh * 512:(qh + 1) * 512],
                                start=(e == 0), stop=(e == NE - 1))
                for j in range(2):
                    o = og * 2 + j
                    for qh in range(NQB):
                        qsl = slice(qh * 512, (qh + 1) * 512)
                        nc.vector.scalar_tensor_tensor(
                            x2_tiles[o][:, qsl], ps[(j, qh)][:], fcb_t[:, o:o + 1],
                            xo[o][:, qsl], op0=OP.add, op1=OP.add)

        # ================= Phase 5: LN2 =================
        _mark(nc, "E:ln2")
        h2_pool = ctx.enter_context(tc.tile_pool(name="h2p", bufs=NE))
        h2_tiles = [h2_pool.tile([128, T_OWN], F16, tag="h2t", name="h2t")
                    for _ in range(NE)]
        with ExitStack() as p5:
            sq_pool = p5.enter_context(tc.tile_pool(name="sq2p", bufs=2))
            st_ps = p5.enter_context(tc.tile_pool(name="st2ps", bufs=2, space="PSUM"))
            row_pool = p5.enter_context(tc.tile_pool(name="rows2", bufs=2))
            map_ps = p5.enter_context(tc.tile_pool(name="map2ps", bufs=2, space="PSUM"))
            stat2_pool = p5.enter_context(tc.tile_pool(name="stat2p", bufs=1))
            srst2 = stat2_pool.tile([1, T_OWN], F32R, tag="srst2")
            snb2 = stat2_pool.tile([2, T_OWN], F32R, tag="snb2")
            nc.sync.dma_start(snb2[1:2, :], ones.ap()[0:T_OWN].unsqueeze(0).bitcast(F32R))
            _ln_stats(nc, cfg, (sq_pool, st_ps, row_pool),
                      lambda e, tb: x2_tiles[e][:, tb * 512:(tb + 1) * 512],
                      NQB, ones_col, eps_t, srst2, snb2)
            for tb in range(NQB):
                sl = slice(tb * 512, (tb + 1) * 512)
                for e in range(NE):
                    _ln_apply(nc, map_ps, ln2t[:, e * 128:(e + 1) * 128],
                              srst2, snb2, x2_tiles[e][:, sl], h2_tiles[e][:, sl], sl)

        # ================= Phase 6: MLP =================
        _mark(nc, "F:mlp1")
        g_pool = ctx.enter_context(tc.tile_pool(name="gp", bufs=NM))
        g_tiles = [g_pool.tile([128, T_OWN], F16, tag="gt", name="gt")
                   for _ in range(NM)]
        with ExitStack() as p6:
            w1_pool = p6.enter_context(tc.tile_pool(name="w1p", bufs=3))
            m1_ps = p6.enter_context(tc.tile_pool(name="m1ps", bufs=3, space="PSUM"))
            for mb in (() if "mlp" in cfg.skip else range(NM // 4)):
                w1b = w1_pool.tile([128, NE, 512], F16, tag="w1b", name="w1b", bufs=2)
                nc.scalar.dma_start(
                    w1b[:], w1T.ap()[:, mb * 512:(mb + 1) * 512].rearrange(
                        "(a p) n -> p a n", p=128))
                w1t = [w1b[:, e, :] for e in range(NE)]
                for j in range(4):
                    m = mb * 4 + j
                    ps = m1_ps.tile([128, T_OWN], F32, tag="m1", name="m1")
                    for qh in range(NQB):
                        qsl = slice(qh * 512, (qh + 1) * 512)
                        for e in range(NE):
                            nc.tensor.matmul(ps[:, qsl], w1t[e][:, j * 128:(j + 1) * 128],
                                             h2_tiles[e][:, qsl],
                                             start=(e == 0), stop=(e == NE - 1))
                    nc.scalar.activation(
                        g_tiles[m][:], ps[:],
                        AF.Gelu, bias=b1_t[:, m:m + 1], scale=1.0)

        _mark(nc, "G:mlp2")
        with ExitStack() as p6b:
            w2_pool = p6b.enter_context(tc.tile_pool(name="w2p", bufs=4))
            out_pool = p6b.enter_context(tc.tile_pool(name="op", bufs=4))
            m2_ps = p6b.enter_context(tc.tile_pool(name="m2ps", bufs=8, space="PSUM"))
            for oh in (() if "mlp" in cfg.skip else range(2)):
                csl = slice(oh * 512, (oh + 1) * 512)
                ps = {(o, qh): m2_ps.tile([128, 512], F32, tag="m2", name="m2")
                      for o in range(4) for qh in range(NQB)}
                for m4 in range(NM // 4):
                    wt = w2_pool.tile([128, 4, 512], F16, tag="w2", name="w2t")
                    nc.scalar.dma_start(
                        wt[:], w2T.ap()[m4 * 512:(m4 + 1) * 512, csl].rearrange(
                            "(a p) n -> p a n", p=128))
                    for a in range(4):
                        m = m4 * 4 + a
                        for o in range(4):
                            for qh in range(NQB):
                                nc.tensor.matmul(
                                    ps[(o, qh)][:], wt[:, a, o * 128:(o + 1) * 128],
                                    g_tiles[m][:, qh * 512:(qh + 1) * 512],
                                    start=(m == 0), stop=(m == NM - 1))
                for o in range(4):
                    of = oh * 4 + o
                    ot = out_pool.tile([128, T_OWN], F32, tag="ot", name="ot")
                    for qh in range(NQB):
                        qsl = slice(qh * 512, (qh + 1) * 512)
                        nc.vector.scalar_tensor_tensor(
                            ot[:, qsl], ps[(o, qh)][:], b2_t[:, of:of + 1],
                            x2_tiles[of][:, qsl], op0=OP.add, op1=OP.add)
                    nc.sync.dma_start(out.ap()[of * 128:(of + 1) * 128, :], ot[:])


# ----------------------------------------------------------------------------
# host driver
# ----------------------------------------------------------------------------
B, S, E_FULL, H_FULL, MLP_FULL = 4, 2048, 1024, 16, 4096
_cache = {}


def _get_nc():
    if "nc" not in _cache:
        _cache["nc"] = build(Cfg())
    return _cache["nc"]


def _host_prepare(x_b, roll, qkv_w, fc_w, fc_b, ln1_g, ln1_b, ln2_g, ln2_b,
                  w1, b1, w2, b2):
    S_, E = x_b.shape
    D = E // H_FULL
    xr = np.roll(x_b, -roll, axis=0)
    qkvT = np.ascontiguousarray(qkv_w.T).astype(np.float32).copy()
    qkvT[:, :E] *= D ** -0.5
    return {
        "xT": np.ascontiguousarray(xr.T).astype(np.float16),
        "qkvT": qkvT.astype(np.float16),
        "fcT": np.ascontiguousarray(fc_w.T).astype(np.float16),
        "w1T": np.ascontiguousarray(w1.T).astype(np.float16),
        "w2T": np.ascontiguousarray(w2.T).astype(np.float16),
        "ln1": np.stack([ln1_g, ln1_b]).astype(np.float32),
        "ln2": np.stack([ln2_g, ln2_b]).astype(np.float32),
        "fcb": np.asarray(fc_b, np.float32),
        "b1": np.asarray(b1, np.float32),
        "b2": np.asarray(b2, np.float32),
        "ones": np.ones((S_,), np.float32),
    }


def kernel(x, qkv_w, fc_w, fc_b, ln1_g, ln1_b, ln2_g, ln2_b, w1, b1, w2, b2):
    from concourse.bass_utils import run_bass_kernel_spmd

    x = np.ascontiguousarray(np.asarray(x, dtype=np.float32))
    args = [np.ascontiguousarray(np.asarray(a, dtype=np.float32)) for a in
            (qkv_w, fc_w, fc_b, ln1_g, ln1_b, ln2_g, ln2_b, w1, b1, w2, b2)]
    nc = _get_nc()
    in_maps = []
    for c in range(8):
        b, hf = c // 2, c % 2
        in_maps.append(_host_prepare(x[b], hf * (S // 2), *args))
    res = run_bass_kernel_spmd(nc, in_maps, list(range(8)))
    out = np.empty((B, S, E_FULL), np.float32)
    for c in range(8):
        b, hf = c // 2, c % 2
        out[b, hf * (S // 2):(hf + 1) * (S // 2), :] = res.results[c]["out"].T
    return out



# revision 42
# speedup vs baseline: 1.9053x; 1.9053x over previous
"""Trainium2 Bass kernel for a dense transformer block, sharded over 8 NeuronCores.

Sharding: core c handles batch b=c//2 and half hf=c%2 of that batch's 2048
tokens ("own" tokens). K/V are computed for the full 2048-token batch on both
cores of a pair, so no collectives are needed.

v3.2: fp8(e4m3) DoubleRow matmuls throughout; LN1 folded into host-side
weights + rank-2 PSUM correction; weights pre-scaled out of the fp8 denormal
range (q x128, k/v/fc/w1 x16, w2 x64) with inverses folded into free scale
slots; Q/K/V projections, q/k strip-remap DMAs and attention software-
pipelined per 4-head group so projection PE work and ACT/DVE softmax-exp
overlap; K/V psum->sbuf copies on ACT with the k-side rstd folded into the
exp scale column; softmax exp split ACT (true Exp) / DVE (Schraudolph
bit-trick straight into fp8 bits); softmax denominators via a ones-column
in V; LN stats from fp8 x via DoubleRow ones-matmuls.
"""

import numpy as np
import ml_dtypes

from contextlib import ExitStack

import concourse.bass as bass
import concourse.bacc as bacc
import concourse.tile as tile
import concourse.mybir as mybir

F32 = mybir.dt.float32
F16 = mybir.dt.float16
F8 = mybir.dt.float8e4
U8 = mybir.dt.uint8
AF = mybir.ActivationFunctionType
OP = mybir.AluOpType
DR = mybir.MatmulPerfMode.DoubleRow

EPS = 1e-5
# Schraudolph fast-exp into fp8e4m3 bits: u8 = round(s * 8*log2(e) + 56)
EXPK = float(8.0 / np.log(2.0))
EXPC = 56.0

PHASE_MARKS = []


def _mark(nc, name):
    PHASE_MARKS.append((name, int(nc.get_next_instruction_name()[2:])))


class Cfg:
    def __init__(self, E=1024, H=16, MLP=4096, T_OWN=1024, T_FULL=2048, repeat=1,
                 skip=(), exp_w=(1, 1)):
        self.E, self.H, self.MLP = E, H, MLP
        self.T_OWN, self.T_FULL = T_OWN, T_FULL
        self.D = E // H
        self.NE = E // 128          # feature tiles
        self.NEP = self.NE // 2     # feature pair-tiles
        self.NM = MLP // 128
        self.NQB = T_OWN // 512     # own-token 512-blocks
        self.NFB = T_FULL // 512    # full-token 512-blocks
        self.NTK = T_FULL // 128    # full-token 128-blocks (k positions)
        self.NTP = self.NTK // 2    # k-position pair-blocks
        self.NG = H // 4            # 4-head groups
        self.repeat = repeat
        self.skip = frozenset(skip)
        self.exp_w = exp_w          # (ACT, DVE) weights for exp split


def build(cfg: Cfg):
    E, MLP, T_OWN, T_FULL = cfg.E, cfg.MLP, cfg.T_OWN, cfg.T_FULL

    nc = bacc.Bacc("TRN2", target_bir_lowering=False, debug=False)

    d = {}
    d["x8"] = nc.dram_tensor("x8", [E, T_FULL], F8, kind="ExternalInput")
    d["xres"] = nc.dram_tensor("xres", [E, T_OWN], F16, kind="ExternalInput")
    d["qkvT"] = nc.dram_tensor("qkvT", [E, 3 * E], F8, kind="ExternalInput")
    d["cqkv"] = nc.dram_tensor("cqkv", [2, 3 * E], F16, kind="ExternalInput")
    d["fcT"] = nc.dram_tensor("fcT", [E, E], F8, kind="ExternalInput")
    d["w1T"] = nc.dram_tensor("w1T", [E, MLP], F8, kind="ExternalInput")
    d["b1c"] = nc.dram_tensor("b1c", [MLP], F32, kind="ExternalInput")
    d["c1m"] = nc.dram_tensor("c1m", [1, MLP], F16, kind="ExternalInput")
    d["w2T"] = nc.dram_tensor("w2T", [MLP, E], F8, kind="ExternalInput")
    d["b2r"] = nc.dram_tensor("b2r", [1, E], F16, kind="ExternalInput")
    d["eye"] = nc.dram_tensor("eye", [128, 128], F16, kind="ExternalInput")
    d["out"] = nc.dram_tensor("out", [E, T_OWN], F32, kind="ExternalOutput")

    PHASE_MARKS.clear()
    with tile.TileContext(nc) as tc, nc.allow_low_precision(
        reason="fp8 matmul inputs by design"
    ):
        if cfg.repeat == 1:
            _body(nc, tc, cfg, d)
        else:
            with tc.For_i(0, cfg.repeat, 1, hint_engines=(
                    mybir.EngineType.PE, mybir.EngineType.Activation,
                    mybir.EngineType.DVE, mybir.EngineType.SP,
                    mybir.EngineType.Pool)):
                _body(nc, tc, cfg, d)
    nc.compile()
    return nc


def _body(nc, tc, cfg, d):
    E, H, MLP = cfg.E, cfg.H, cfg.MLP
    NE, NEP, NM = cfg.NE, cfg.NEP, cfg.NM
    NQB, NFB, NTK, NTP, NG = cfg.NQB, cfg.NFB, cfg.NTK, cfg.NTP, cfg.NG
    T_OWN, T_FULL = cfg.T_OWN, cfg.T_FULL

    with ExitStack() as ctx:
        consts = ctx.enter_context(tc.tile_pool(name="consts", bufs=1))

        ones8_col = consts.tile([128, 2, 16], F8)
        nc.vector.memset(ones8_col[:], 0.0)
        nc.vector.memset(ones8_col[:, :, 0:1], 1.0)
        ones_row = consts.tile([1, 128], F16)
        nc.vector.memset(ones_row[:], 1.0)
        t32_row = consts.tile([1, 64], F16)
        nc.vector.memset(t32_row[:], 32.0)
        sixt_row = consts.tile([1, 128], F16)
        nc.vector.memset(sixt_row[:], 1.0 / 16.0)
        ones512 = consts.tile([1, 512], F16)
        nc.vector.memset(ones512[:], 1.0)
        ones_col = consts.tile([128, 1], F16)
        nc.vector.memset(ones_col[:], 1.0)
        eps_t = consts.tile([1, 1], F32)
        nc.vector.memset(eps_t[:], EPS)
        eye_t = consts.tile([128, 128], F16)
        nc.sync.dma_start(eye_t[:], d["eye"].ap())
        b1_t = consts.tile([128, NM], F32)
        nc.sync.dma_start(b1_t[:], d["b1c"].ap().rearrange("(a p) -> p a", p=128))
        b2r_t = consts.tile([1, E], F16)
        nc.sync.dma_start(b2r_t[:], d["b2r"].ap())

        # ------- right-stack pools, freed mid-kernel (alloc in reverse
        # free order) -------
        av_stack = ExitStack()  # freed after fc
        av_pool = av_stack.enter_context(
            tc.tile_pool(name="avp", bufs=1, side="right"))
        av8 = [av_pool.tile([64, 2, T_OWN], F8, name="av8", tag="av8", bufs=H // 2)
               for _ in range(H // 2)]

        att_stack = ExitStack()  # freed after attention
        att_pool = att_stack.enter_context(
            tc.tile_pool(name="attp", bufs=1, side="right"))
        q8s = [att_pool.tile([128, 2, T_OWN], F8, name="q8s", tag="q8s", bufs=NG)
               for _ in range(NG)]
        k8s = [att_pool.tile([128, 2, T_FULL], F8, name="k8s", tag="k8s", bufs=NG)
               for _ in range(NG)]
        v8 = att_pool.tile([128, NTP, 2, H * 65], F8, name="v8")

        xp = ExitStack()  # x8/wqkv/q8f/k8f/rows live through projections only
        qkf_pool = xp.enter_context(tc.tile_pool(name="qkf", bufs=1, side="right"))
        q8f = qkf_pool.tile([128, NE, T_OWN], F8, name="q8f")
        k8f = qkf_pool.tile([128, NE, T_FULL], F8, name="k8f")
        x_pool = xp.enter_context(tc.tile_pool(name="xp", bufs=1, side="right"))
        x8_t = x_pool.tile([128, NE, T_FULL], F8, name="x8t")
        wqkv = x_pool.tile([128, NEP, 2, 3 * E], F8, name="wqkv")
        cqkv_t = x_pool.tile([2, 3 * E], F16, name="cqkvt")
        nc.sync.dma_start(cqkv_t[:], d["cqkv"].ap())
        rows1 = x_pool.tile([2, T_FULL], F16, name="rows1")   # -mu, sd
        rows3 = x_pool.tile([3, T_FULL], F16, name="rows3")   # rstd/16,rstd,rstd*K
        rstd1 = x_pool.tile([1, T_FULL], F16, name="rstd1")

        xr_pool = ctx.enter_context(tc.tile_pool(name="xrp", bufs=1))
        xres_t = xr_pool.tile([128, NE, T_OWN], F16, name="xrest")
        rcol_pool = ctx.enter_context(tc.tile_pool(name="rcolp", bufs=1))
        rcols = rcol_pool.tile([128, NTK, 3], F32, name="rcols")
        bc_pool = ctx.enter_context(tc.tile_pool(name="bcp", bufs=1))
        bc16 = bc_pool.tile([128, NFB, 512], F16, name="bc16")  # bcast(rstd/16)

        nc.gpsimd.memset(v8[:].rearrange("p a b (h e) -> p (a b h) e", e=65)
                         [:, :, 64:65], 1.0)

        # DMA inputs (x8 split per token block so stats start early; wqkv
        # split by q/k/v section, k first since k_chunks run first)
        for tb in range(NFB):
            sl = slice(tb * 512, (tb + 1) * 512)
            nc.scalar.dma_start(
                x8_t[:, :, sl],
                d["x8"].ap()[:, sl].rearrange("(a p) n -> p a n", p=128))
            wsl = slice(((tb + 1) % 3) * E, ((tb + 1) % 3 + 1) * E)
            if tb < 3:
                nc.gpsimd.dma_start(
                    wqkv[:, :, :, wsl],
                    d["qkvT"].ap()[:, wsl].rearrange(
                        "(a two p) n -> p a two n", p=128, two=2))

        # ============ Phase A1: LN1 stats (from fp8 x, DoubleRow ones) =====
        _mark(nc, "A:ln1")
        pA = ExitStack()
        sq_pool = pA.enter_context(tc.tile_pool(name="sqp", bufs=4))
        st_ps = pA.enter_context(tc.tile_pool(name="stps", bufs=2, space="PSUM"))
        row_pool = pA.enter_context(tc.tile_pool(name="rows", bufs=2))
        bc_ps = pA.enter_context(tc.tile_pool(name="bcps", bufs=1, space="PSUM"))
        tcol_ps = pA.enter_context(tc.tile_pool(name="tcolps", bufs=2, space="PSUM"))
        for tb in range(NFB):
            sl = slice(tb * 512, (tb + 1) * 512)
            s1 = st_ps.tile([16, 512], F32, tag="s1", name="s1")
            s2 = st_ps.tile([16, 512], F32, tag="s2", name="s2")
            for ep in range(NEP):
                sq = sq_pool.tile([128, 2, 512], F8, tag="sq", name="sq")
                if ep % 2 == 0:
                    nc.scalar.activation(sq[:], x8_t[:, 2 * ep:2 * ep + 2, sl],
                                         AF.Square)
                else:
                    nc.gpsimd.tensor_tensor(sq[:], x8_t[:, 2 * ep:2 * ep + 2, sl],
                                            x8_t[:, 2 * ep:2 * ep + 2, sl],
                                            OP.mult)
                nc.tensor.matmul(s1[:], ones8_col[:],
                                 x8_t[:, 2 * ep:2 * ep + 2, sl],
                                 start=(ep == 0), stop=(ep == NEP - 1),
                                 perf_mode=DR)
                nc.tensor.matmul(s2[:], ones8_col[:], sq[:],
                                 start=(ep == 0), stop=(ep == NEP - 1),
                                 perf_mode=DR)
            # row stats: -mu, sd, rstd + scaled variants
            nc.vector.tensor_scalar(rows1[0:1, sl], s1[0:1, :], -1.0 / E, None,
                                    OP.mult)
            msq = row_pool.tile([1, 512], F32, tag="msq", name="msq")
            nc.vector.tensor_tensor(msq[:], rows1[0:1, sl], rows1[0:1, sl], OP.mult)
            var = row_pool.tile([1, 512], F32, tag="var", name="var")
            nc.vector.scalar_tensor_tensor(var[:], s2[0:1, :], 1.0 / E, msq[:],
                                           op0=OP.mult, op1=OP.subtract)
            sd = row_pool.tile([1, 512], F16, tag="sd", name="sd")
            nc.scalar.activation(sd[:], var[:], AF.Sqrt, bias=eps_t[:], scale=1.0)
            nc.gpsimd.dma_start(rows1[1:2, sl], sd[:])
            nc.vector.reciprocal(rstd1[0:1, sl], sd[:])
            nc.vector.tensor_scalar(rows3[0:1, sl], rstd1[0:1, sl], 1.0 / 16.0,
                                    None, OP.mult)
            rek = row_pool.tile([1, 512], F16, tag="rek", name="rek")
            nc.vector.tensor_scalar(rek[:], rstd1[0:1, sl], EXPK, None, OP.mult)
            nc.sync.dma_start(rows3[1:2, sl], rstd1[0:1, sl])
            nc.scalar.dma_start(rows3[2:3, sl], rek[:])
            # q-normalize broadcast: bcast(rstd/16) -> sbuf f16
            bc = bc_ps.tile([128, 512], F32, tag="bc", name="bc")
            nc.tensor.matmul(bc[:], sixt_row[:], rstd1[0:1, sl],
                             start=True, stop=True)
            nc.vector.tensor_copy(bc16[:, tb, :], bc[:])
            # per-tk transposed scale columns [rstd/16, rstd, rstd*EXPK]
            for tk in range(4 * tb, 4 * tb + 4):
                tsl = slice(tk * 128, (tk + 1) * 128)
                pcol = tcol_ps.tile([128, 3], F16, tag="pcol", name="pcol")
                nc.tensor.transpose(pcol[:], rows3[:, tsl], eye_t[0:3, 0:3])
                nc.vector.tensor_copy(rcols[:, tk, :], pcol[:])
        pA.close()

        # ====== Phase A2+D: projections / remap / attention, interleaved ===
        _mark(nc, "D:att")
        wa, wd = cfg.exp_w[0], cfg.exp_w[1]
        cnt = [0.0, 0.0]
        exp_sched = []
        for i in range(256):
            fr = [(cnt[0] + 1) / wa, (cnt[1] + 1) / wd]
            j = 0 if fr[0] <= fr[1] else 1
            cnt[j] += 1
            exp_sched.append("AD"[j])
        exp_i = [0]

        pD = ExitStack()
        sc_ps = pD.enter_context(tc.tile_pool(name="scps", bufs=4, space="PSUM"))
        av_ps = pD.enter_context(tc.tile_pool(name="avps", bufs=2, space="PSUM"))
        ex_pool = pD.enter_context(tc.tile_pool(name="exp", bufs=8))
        rec_pool = pD.enter_context(tc.tile_pool(name="recp", bufs=4))

        def k_chunk(o, tb):
            sl = slice(tb * 512, (tb + 1) * 512)
            osl = slice(E + o * 128, E + (o + 1) * 128)
            ps = sc_ps.tile([128, 512], F32, tag="sc", name="acc")
            for ep in range(NEP):
                nc.tensor.matmul(ps[:], wqkv[:, ep, :, osl],
                                 x8_t[:, 2 * ep:2 * ep + 2, sl],
                                 start=(ep == 0), stop=False, perf_mode=DR)
            nc.tensor.matmul(ps[:], cqkv_t[:, osl], rows1[:, sl],
                             start=False, stop=True)
            nc.scalar.activation(k8f[:, o, sl], ps[:], AF.Copy, scale=1.0 / 16.0)

        def q_chunk(o, tb):
            sl = slice(tb * 512, (tb + 1) * 512)
            osl = slice(o * 128, (o + 1) * 128)
            ps = sc_ps.tile([128, 512], F32, tag="sc", name="acc")
            for ep in range(NEP):
                nc.tensor.matmul(ps[:], wqkv[:, ep, :, osl],
                                 x8_t[:, 2 * ep:2 * ep + 2, sl],
                                 start=(ep == 0), stop=False, perf_mode=DR)
            nc.tensor.matmul(ps[:], cqkv_t[:, osl], rows1[:, sl],
                             start=False, stop=True)
            nc.vector.scalar_tensor_tensor(q8f[:, o, sl], ps[:], 1.0 / 8.0,
                                           bc16[:, tb, :], op0=OP.mult,
                                           op1=OP.mult)

        def v_chunk(g, tk):
            tsl = slice(tk * 128, (tk + 1) * 128)
            vsl = slice(2 * E + g * 256, 2 * E + (g + 1) * 256)
            ps = sc_ps.tile([128, 512], F32, tag="sc", name="acc")
            for ep in range(NEP):
                nc.tensor.matmul(ps[:, 0:256], x8_t[:, 2 * ep:2 * ep + 2, tsl],
                                 wqkv[:, ep, :, vsl],
                                 start=(ep == 0), stop=False, perf_mode=DR)
            nc.tensor.matmul(ps[:, 0:256], rows1[:, tsl], cqkv_t[:, vsl],
                             start=False, stop=True)
            dst = v8[:, tk // 2, tk % 2,
                     g * 260:(g + 1) * 260].rearrange(
                         "p (h e) -> p h e", e=65)[:, :, 0:64]
            nc.scalar.activation(dst, ps[:, 0:256].rearrange(
                "p (h e) -> p h e", e=64), AF.Copy, scale=rcols[:, tk, 0:1])

        def remap_head(h):
            g, s = h // 4, h % 4
            for c in range(2):
                src_p = 64 * (h % 2) + 32 * c
                nc.sync.dma_start(k8s[g][32 * s:32 * s + 32, c, :],
                                  k8f[src_p:src_p + 32, h // 2, :])
                nc.scalar.dma_start(q8s[g][32 * s:32 * s + 32, c, :],
                                    q8f[src_p:src_p + 32, h // 2, :])

        def proj_chunks(g):
            for o in (2 * g, 2 * g + 1):
                if "kv" not in cfg.skip:
                    for tb in range(NFB):
                        yield lambda o=o, tb=tb: k_chunk(o, tb)
                if "q" not in cfg.skip:
                    for tb in range(NQB):
                        yield lambda o=o, tb=tb: q_chunk(o, tb)
                if "att" not in cfg.skip:
                    yield lambda o=o: remap_head(2 * o)
                    yield lambda o=o: remap_head(2 * o + 1)
            if "kv" not in cfg.skip:
                for tk in range(NTK):
                    yield lambda tk=tk: v_chunk(g, tk)

        avp_tiles = {}

        def att_scores(g, h, s, tp):
            ssl = slice(32 * s, 32 * s + 32)
            if tp == 0:
                avp_tiles[h] = av_ps.tile([65, T_OWN], F32, tag="av", name="av")
            ex = ex_pool.tile([128, 2, T_OWN], U8, tag="ex", name="ex")
            for ti in range(2):
                tk = 2 * tp + ti
                ksl = slice(tk * 128, (tk + 1) * 128)
                for qh in range(NQB):
                    qsl = slice(qh * 512, (qh + 1) * 512)
                    sc = sc_ps.tile([128, 512], F32, tag="sc", name="sc")
                    nc.tensor.matmul(sc[:], k8s[g][ssl, :, ksl],
                                     q8s[g][ssl, :, qsl],
                                     start=True, stop=True, perf_mode=DR,
                                     tile_position=(32 * s, 0))
                    eng = exp_sched[exp_i[0] % 256]
                    exp_i[0] += 1
                    dst = ex[:, ti, qsl]
                    if eng == "A" or "fastexp" in cfg.skip:
                        nc.scalar.activation(dst.bitcast(F8), sc[:], AF.Exp,
                                             scale=rcols[:, tk, 1:2])
                    else:
                        nc.vector.tensor_scalar(dst, sc[:], rcols[:, tk, 2:3],
                                                EXPC, OP.mult, OP.add)
            return ex

        def att_av(h, tp, ex):
            avp = avp_tiles[h]
            vsl = slice(h * 65, (h + 1) * 65)
            for qh in range(NQB):
                qsl = slice(qh * 512, (qh + 1) * 512)
                nc.tensor.matmul(avp[:, qsl], v8[:, tp, :, vsl],
                                 ex[:, :, qsl].bitcast(F8),
                                 start=(tp == 0), stop=(tp == NTP - 1),
                                 perf_mode=DR)

        def att_final(h, hp):
            # NB: DVE/ACT may read only ONE operand from PSUM, so the rm
            # broadcast is staged through SBUF (on ACT; DVE does the multiply).
            avp = avp_tiles.pop(h)
            rrow = rec_pool.tile([1, T_OWN], F16, tag="rr", name="rr")
            nc.vector.reciprocal(rrow[:], avp[64:65, :])
            for qh in range(NQB):
                qsl = slice(qh * 512, (qh + 1) * 512)
                rm = sc_ps.tile([64, 512], F32, tag="sc", name="rm")
                nc.tensor.matmul(rm[:], t32_row[:], rrow[0:1, qsl],
                                 start=True, stop=True)
                rm16 = rec_pool.tile([64, 512], F16, tag="rm16", name="rm16")
                nc.scalar.activation(rm16[:], rm[:], AF.Copy)
                nc.vector.tensor_tensor(av8[hp][:, h % 2, qsl], avp[0:64, qsl],
                                        rm16[:], OP.mult)

        pend = []  # deferred AV/finalize closures, drained one per unit

        def att_units(g):
            # Emit units with the AV matmuls deferred one unit and the
            # per-head finalize deferred into the next head's stream, so the
            # in-order PE queue never blocks on just-issued exps. The pend
            # queue carries across heads AND groups; drained at driver end.
            if "att" in cfg.skip:
                return []

            def unit(g, h, s, tp):
                def run():
                    if pend:
                        pend.pop(0)()
                    ex = att_scores(g, h, s, tp)
                    pend.append(lambda: att_av(h, tp, ex))
                return run

            def head_fin(h):
                def run():
                    pend.append(lambda: att_final(h, h // 2))
                return run

            units = []
            for s in range(4):
                h = 4 * g + s
                for tp in range(NTP):
                    units.append(unit(g, h, s, tp))
                units.append(head_fin(h))
            return units

        # driver: group 0 projections first, then interleave att(g-1)+proj(g)
        for c in proj_chunks(0):
            c()
        for g in range(1, NG + 1):
            units = att_units(g - 1)
            chunks = list(proj_chunks(g)) if g < NG else []
            ci = 0
            for i, u in enumerate(units):
                u()
                tgt = (i + 1) * len(chunks) // max(1, len(units))
                while ci < tgt:
                    chunks[ci]()
                    ci += 1
            while ci < len(chunks):
                chunks[ci]()
                ci += 1
        while pend:
            pend.pop(0)()
        if "att" in cfg.skip:
            for t in av8:
                nc.vector.memset(t[:, :, 0:1], 0.0)
        pD.close()
        xp.close()
        att_stack.close()

        nc.gpsimd.dma_start(
            xres_t[:], d["xres"].ap().rearrange("(a p) n -> p a n", p=128))

        # late weight pools (allocated after the big right-stack frees)
        wf_pool = ctx.enter_context(tc.tile_pool(name="wfp", bufs=1))
        wf = wf_pool.tile([64, NE, 2, E], F8, name="wf")
        nc.scalar.dma_start(
            wf[:], d["fcT"].ap().rearrange("(a two p) n -> p a two n", p=64, two=2))
        w1_pool = ctx.enter_context(tc.tile_pool(name="w1p", bufs=1))
        w1t = w1_pool.tile([128, NEP, 2, MLP], F8, name="w1t")
        nc.gpsimd.dma_start(
            w1t[:], d["w1T"].ap().rearrange("(a two p) n -> p a two n", p=128, two=2))

        # ============ Phase E: fc_out + residual ============
        _mark(nc, "E:fc")
        x2_pool = ctx.enter_context(tc.tile_pool(name="x2p", bufs=1))
        x2_t = x2_pool.tile([128, NE, T_OWN], F16, name="x2t")
        with ExitStack() as pE:
            fc_ps = pE.enter_context(tc.tile_pool(name="fcps", bufs=6, space="PSUM"))
            if "fc" in cfg.skip:
                nc.vector.tensor_copy(x2_t[:], xres_t[:])
            for qh in (() if "fc" in cfg.skip else range(NQB)):
                qsl = slice(qh * 512, (qh + 1) * 512)
                for o in range(NE):
                    osl = slice(o * 128, (o + 1) * 128)
                    ps = fc_ps.tile([128, 512], F32, tag="fc", name="fc")
                    for p in range(H // 2):
                        nc.tensor.matmul(ps[:], wf[:, p, :, osl], av8[p][:, :, qsl],
                                         start=(p == 0), stop=(p == H // 2 - 1),
                                         perf_mode=DR)
                    nc.vector.scalar_tensor_tensor(
                        x2_t[:, o, qsl], ps[:], 1.0 / 512.0, xres_t[:, o, qsl],
                        op0=OP.mult, op1=OP.add)
        av_stack.close()

        # ============ Phase F: LN2 ============
        _mark(nc, "F:ln2")
        h2_pool = ctx.enter_context(tc.tile_pool(name="h2p", bufs=1))
        h28 = h2_pool.tile([128, NEP, 2, T_OWN], F8, name="h28")
        with ExitStack() as pF:
            sq_pool2 = pF.enter_context(tc.tile_pool(name="sq2p", bufs=4))
            st_ps2 = pF.enter_context(tc.tile_pool(name="st2ps", bufs=2, space="PSUM"))
            row_pool2 = pF.enter_context(tc.tile_pool(name="rows2", bufs=2))
            bc_ps2 = pF.enter_context(tc.tile_pool(name="bc2ps", bufs=2, space="PSUM"))
            t_pool = pF.enter_context(tc.tile_pool(name="t2p", bufs=4))
            rows2 = row_pool2.tile([2, T_OWN], F16, tag="rows2", name="rows2")
            rstd2 = row_pool2.tile([1, T_OWN], F16, tag="rstd2", name="rstd2")
            for tb in range(NQB):
                sl = slice(tb * 512, (tb + 1) * 512)
                s1 = st_ps2.tile([1, 512], F32, tag="s1", name="s1")
                s2 = st_ps2.tile([1, 512], F32, tag="s2", name="s2")
                for e in range(NE):
                    sq = sq_pool2.tile([128, 512], F16, tag="sq", name="sq")
                    if e % 2 == 0:
                        nc.scalar.activation(sq[:], x2_t[:, e, sl], AF.Square)
                    else:
                        nc.gpsimd.tensor_tensor(sq[:], x2_t[:, e, sl],
                                                x2_t[:, e, sl], OP.mult)
                    nc.tensor.matmul(s1[:], ones_col[:], x2_t[:, e, sl],
                                     start=(e == 0), stop=(e == NE - 1))
                    nc.tensor.matmul(s2[:], ones_col[:], sq[:],
                                     start=(e == 0), stop=(e == NE - 1))
                nc.vector.tensor_scalar(rows2[0:1, sl], s1[:], -1.0 / E, None,
                                        OP.mult)
                msq = row_pool2.tile([1, 512], F32, tag="msq", name="msq")
                nc.vector.tensor_tensor(msq[:], rows2[0:1, sl], rows2[0:1, sl],
                                        OP.mult)
                var = row_pool2.tile([1, 512], F32, tag="var", name="var")
                nc.vector.scalar_tensor_tensor(var[:], s2[:], 1.0 / E, msq[:],
                                               op0=OP.mult, op1=OP.subtract)
                sd = row_pool2.tile([1, 512], F16, tag="sd", name="sd")
                nc.scalar.activation(sd[:], var[:], AF.Sqrt, bias=eps_t[:],
                                     scale=1.0)
                nc.vector.reciprocal(rstd2[0:1, sl], sd[:])
                nmu_bc = bc_ps2.tile([128, 512], F32, tag="nmu", name="nmu")
                nc.tensor.matmul(nmu_bc[:], ones_row[:], rows2[0:1, sl],
                                 start=True, stop=True)
                rs_bc = bc_ps2.tile([128, 512], F32, tag="rs", name="rs")
                nc.tensor.matmul(rs_bc[:], ones_row[:], rstd2[0:1, sl],
                                 start=True, stop=True)
                for e in range(NE):
                    t16 = t_pool.tile([128, 512], F16, tag="t16", name="t16")
                    nc.vector.tensor_tensor(t16[:], x2_t[:, e, sl], nmu_bc[:],
                                            OP.add)
                    nc.vector.tensor_tensor(h28[:, e // 2, e % 2, sl], t16[:],
                                            rs_bc[:], OP.mult)

        # ============ Phase G/H: MLP ============
        _mark(nc, "G:mlp1")
        g_pool = ctx.enter_context(tc.tile_pool(name="gp", bufs=1))
        g8 = g_pool.tile([128, NM // 2, 2, T_OWN], F8, name="g8")
        w2_pool = ctx.enter_context(tc.tile_pool(name="w2p", bufs=1))
        w2t = w2_pool.tile([128, NM // 2, 2, E], F8, name="w2t")
        nc.gpsimd.dma_start(
            w2t[:], d["w2T"].ap().rearrange("(a two p) n -> p a two n", p=128, two=2))
        with ExitStack() as pG:
            m1_ps = pG.enter_context(tc.tile_pool(name="m1ps", bufs=3, space="PSUM"))
            for m in (() if "mlp" in cfg.skip else range(NM)):
                msl = slice(m * 128, (m + 1) * 128)
                ps = m1_ps.tile([128, T_OWN], F32, tag="m1", name="m1")
                for qh in range(NQB):
                    qsl = slice(qh * 512, (qh + 1) * 512)
                    for ep in range(NEP):
                        nc.tensor.matmul(ps[:, qsl], w1t[:, ep, :, msl],
                                         h28[:, ep, :, qsl],
                                         start=(ep == 0), stop=(ep == NEP - 1),
                                         perf_mode=DR)
                nc.scalar.activation(g8[:, m // 2, m % 2, :], ps[:], AF.Gelu,
                                     bias=b1_t[:, m:m + 1], scale=1.0 / 16.0)

        _mark(nc, "H:mlp2")
        with ExitStack() as pH:
            m2_ps = pH.enter_context(tc.tile_pool(name="m2ps", bufs=8, space="PSUM"))
            out_pool = pH.enter_context(tc.tile_pool(name="op", bufs=4))
            for wave in (() if "mlp" in cfg.skip else range(2)):
                ps = {(o, qh): m2_ps.tile([128, 512], F32, tag="m2", name="m2")
                      for o in range(NE // 2) for qh in range(NQB)}
                for mp in range(NM // 2):
                    for o in range(NE // 2):
                        of = wave * (NE // 2) + o
                        osl = slice(of * 128, (of + 1) * 128)
                        for qh in range(NQB):
                            qsl = slice(qh * 512, (qh + 1) * 512)
                            nc.tensor.matmul(
                                ps[(o, qh)][:], w2t[:, mp, :, osl],
                                g8[:, mp, :, qsl],
                                start=(mp == 0), stop=False,
                                perf_mode=DR)
                for o in range(NE // 2):
                    of = wave * (NE // 2) + o
                    osl = slice(of * 128, (of + 1) * 128)
                    ot = out_pool.tile([128, T_OWN], F32, tag="ot", name="ot")
                    for qh in range(NQB):
                        qsl = slice(qh * 512, (qh + 1) * 512)
                        nc.tensor.matmul(ps[(o, qh)][:], b2r_t[0:1, osl],
                                         ones512[:], start=False, stop=True)
                        nc.vector.scalar_tensor_tensor(
                            ot[:, qsl], ps[(o, qh)][:], 1.0 / 64.0,
                            x2_t[:, of, qsl], op0=OP.mult, op1=OP.add)
                    nc.sync.dma_start(d["out"].ap()[of * 128:(of + 1) * 128, :],
                                      ot[:])
            if "mlp" in cfg.skip:
                for of in range(NE):
                    ot = out_pool.tile([128, T_OWN], F32, tag="ot", name="ot")
                    nc.vector.tensor_copy(ot[:], x2_t[:, of, :])
                    nc.sync.dma_start(d["out"].ap()[of * 128:(of + 1) * 128, :],
                                      ot[:])


# ----------------------------------------------------------------------------
# host driver
# ----------------------------------------------------------------------------
B, S, E_FULL, H_FULL, MLP_FULL = 4, 2048, 1024, 16, 4096
_cache = {}


def _fp8(a):
    return np.clip(np.asarray(a, np.float32), -240.0, 240.0).astype(
        ml_dtypes.float8_e4m3)


def _get_nc():
    if "nc" not in _cache:
        _cache["nc"] = build(Cfg())
    return _cache["nc"]


def _host_prepare(x_b, roll, qkv_w, fc_w, fc_b, ln1_g, ln1_b, ln2_g, ln2_b,
                  w1, b1, w2, b2):
    S_, E = x_b.shape
    D = E // H_FULL
    T_OWN = S_ // 2
    xr = np.roll(np.asarray(x_b, np.float32), -roll, axis=0)
    xT = np.ascontiguousarray(xr.T)
    Wq = np.asarray(qkv_w, np.float32).copy()
    Wq[:E] *= D ** -0.5
    Wg = Wq * np.asarray(ln1_g, np.float32)[None, :]
    c1 = Wq @ np.asarray(ln1_g, np.float32)
    c2 = Wq @ np.asarray(ln1_b, np.float32)
    W1g = np.asarray(w1, np.float32) * np.asarray(ln2_g, np.float32)[None, :]
    # fp8 denormal avoidance: q cols x128, k/v cols x16 (inverses folded into
    # the kernel's normalize scalars); fc/w1 x16, w2 x64.
    qsc = np.concatenate([np.full(E, 128.0), np.full(2 * E, 16.0)]).astype(
        np.float32)
    return {
        "x8": _fp8(xT),
        "xres": (xT[:, :T_OWN] + np.asarray(fc_b, np.float32)[:, None]).astype(
            np.float16),
        "qkvT": _fp8(Wg.T * qsc[None, :]),
        "cqkv": (np.stack([c1, c2]) * qsc[None, :]).astype(np.float16),
        "fcT": _fp8(np.asarray(fc_w, np.float32).T * 16.0),
        "w1T": _fp8(W1g.T * 16.0),
        "b1c": (np.asarray(b1, np.float32)
                + np.asarray(w1, np.float32) @ np.asarray(ln2_b, np.float32)),
        "c1m": (W1g.sum(axis=1) * 16.0).astype(np.float16)[None, :],
        "w2T": _fp8(np.asarray(w2, np.float32).T * 64.0),
        "b2r": (np.asarray(b2, np.float32) * 64.0).astype(np.float16)[None, :],
        "eye": np.eye(128, dtype=np.float16),
    }


def kernel(x, qkv_w, fc_w, fc_b, ln1_g, ln1_b, ln2_g, ln2_b, w1, b1, w2, b2):
    from concourse.bass_utils import run_bass_kernel_spmd

    x = np.ascontiguousarray(np.asarray(x, dtype=np.float32))
    args = [np.ascontiguousarray(np.asarray(a, dtype=np.float32)) for a in
            (qkv_w, fc_w, fc_b, ln1_g, ln1_b, ln2_g, ln2_b, w1, b1, w2, b2)]
    nc = _get_nc()
    in_maps = []
    for c in range(8):
        b, hf = c // 2, c % 2
        in_maps.append(_host_prepare(x[b], hf * (S // 2), *args))
    res = run_bass_kernel_spmd(nc, in_maps, list(range(8)))
    out = np.empty((B, S, E_FULL), np.float32)
    for c in range(8):
        b, hf = c // 2, c % 2
        out[b, hf * (S // 2):(hf + 1) * (S // 2), :] = res.results[c]["out"].T
    return out
